# revision 2
# baseline (speedup 1.0000x reference)
"""Causal self-attention (RoPE) Trainium2 kernel, v2.

Model: B=2, T=2048, D=2048, 16 heads x 128 head-dim, RoPE theta=1e4.

Sharding (8 cores): cores 0-3 own batch 0, cores 4-7 own batch 1; within a
batch group each core owns 4 heads. Each core computes QKV for its heads,
runs causal attention, and produces a partial output projection (its head
rows of w_out); the host sums 4 partials per batch.

v2 strategy (vs the transpose-heavy v1):
- Q^T/K^T are computed DIRECTLY in [head-dim, T] orientation as W^T @ x^T
  (bf16 weight chunks stationary, x^T streamed 1024 wide) - no PE
  transposes, and bf16 stationary operands load in 1 pass (f32r takes 4).
- RoPE in transposed layout. rotate_half is a partition-halves swap, which
  DVE cannot do (no cross-lane path; the device compiler rejects it), so
  the swap is one PE matmul with a constant permutation. The sin table is
  pre-swapped on the host so the elementwise sin-mul happens BEFORE the
  swap: u = q*sin_preswap (all-bf16 SBUF, 4x DVE mode), swap(u) via PE,
  then one 1x add against the swap's PSUM output.
- Causal mask is multiplicative {0,1}-bf16 applied to exp() output (4x DVE)
  instead of additive -1e6 on f32 PSUM logits (1x DVE).
- Softmax denominators accumulate on DVE (f32); one ones-matmul per
  (head, q-chunk) reduces partitions, reciprocal is broadcast back through
  a rank-1 PE matmul.
- Phase-3 (out-proj) t-tiles are emitted interleaved into the NEXT
  q-chunk's attention loop so PE has dense matmul work while ACT chews
  through exp()s.
- All attention internals bf16: vs a float64 oracle this adds nothing over
  the bf16 input quantization (rel err stays ~3.5e-3).
"""

import sys

sys.path.insert(0, "/opt/trn_rl_repo")

import numpy as np

import concourse.bass as bass
import concourse.mybir as mybir
from concourse import tile
from concourse.bass_utils import run_bass_kernel_spmd

F32 = mybir.dt.float32
F32R = mybir.dt.float32r
BF16 = mybir.dt.bfloat16
F16 = mybir.dt.float16
AF = mybir.ActivationFunctionType

B, T, D = 2, 2048, 2048
H, HD = 16, 128
N_CORES = 8
GROUPS = 2                   # batch groups
CPG = N_CORES // GROUPS      # cores per group (4)
HPC = H // CPG               # heads per core (4)
DL = HPC * HD                # local head dims (512)
ROPE_THETA = 10000.0
SCALE = float(HD) ** -0.5

KD_N = D // 128              # 16 contraction chunks over d_model
TC_N = 2                     # t-chunks of 1024 in phase 1
QC_N = T // 512              # 4 q-chunks of 512
KI_N = T // 128              # 16 k-tiles
NC_N = D // 512              # 4 n-chunks for the output projection

SWAP_MODE = "dma"            # "dma": SBUF->SBUF partition swap; "pe": matmul
QC_ORDER = [0, 1, 2, 3]      # reversed order measured the same on HW

# blob layout (bf16 elements); regions ordered by first use so compute can
# start while later regions are still in flight
WK_OFF = 0
WKQ_N = D * DL                        # [128, 4, 16, 128] per side (h-major)
XT_OFF = WK_OFF + WKQ_N
XT_N = D * T                          # 2 chunks of [128, 16, 1024]
WQ_OFF = XT_OFF + XT_N
COST_OFF = WQ_OFF + WKQ_N
COST_N = HD * T                       # [128, 2048] bf16
SINS_OFF = COST_OFF + COST_N
SINS_N = HD * T                       # pre-swapped sin, bf16
WV_OFF = SINS_OFF + SINS_N
WV_N = D * DL                         # [128, 16, 512]
WOUT_OFF = WV_OFF + WV_N
WOUT_N = DL * D                       # [128, 4, 2048]
MASK_OFF = WOUT_OFF + WOUT_N
MASK_N = 4 * 128 * 512                # [128, 4, 512] {0,1} bf16
SWT_OFF = MASK_OFF + MASK_N
SWT_N = 128 * 128                     # [128, 128] swap permutation
BLOB_N = SWT_OFF + SWT_N


def _split_multi_waits(nc):
    """This container's walrus accepts at most ONE semaphore wait per
    instruction; hoist extra waits onto single-wait NoOps inserted right
    before the instruction on the same engine (sequencers run in order, so
    semantics are unchanged)."""
    n = 0
    for f in nc.m.functions:
        for b in f.blocks:
            il = b.instructions
            if not any(
                i.sync_info is not None and len(i.sync_info.on_wait) > 1
                for i in il
            ):
                continue
            out = []
            for inst in il:
                si = inst.sync_info
                if si is not None and len(si.on_wait) > 1:
                    waits = list(si.on_wait)
                    for w in waits[:-1]:
                        nop = mybir.InstNoOp(
                            name=nc.get_next_instruction_name(), ins=[], outs=[]
                        )
                        nop.engine = inst.engine
                        nop.sync_info = mybir.SyncInfo(on_wait=[w], on_update=[])
                        nc.register_instruction(nop)
                        out.append(nop)
                        n += 1
                    inst.sync_info = mybir.SyncInfo(
                        on_wait=[waits[-1]], on_update=list(si.on_update)
                    )
                out.append(inst)
            il[:] = out
    return n


def _emit_body(nc, tc, io, stk, stop_after=None):
    blob = io["blob"]
    persist = stk.enter_context(tc.tile_pool(name="persist", bufs=1))

    # qT/kT: [head-dim part, head, t] bf16; v_res: [t part, ktile, head*d]
    qT = persist.tile([128, HPC, T], BF16, name="qT")
    kT = persist.tile([128, HPC, T], BF16, name="kT")
    v_res = persist.tile([128, KI_N, DL], BF16, name="v_res")
    ones = persist.tile([128, 128], F32, name="ones")  # f32r memset: no ISA
    nc.vector.memset(ones[:], 1.0)
    ones_r = ones[0:1, 0:128].bitcast(F32R)
    # fp16 ones column: walrus requires matmul operand transfer types to
    # match when either is f32/f32r, so the fp16-sums reduce needs its own
    ones16 = persist.tile([128, 1], F16, name="ones16")
    nc.vector.tensor_copy(ones16[:], ones[0:128, 0:1])  # fp16 memset: no ISA
    ones_c = ones16[0:128, 0:1]
    wout = persist.tile([128, HPC, D], BF16, name="wout")
    masks = persist.tile([128, 4, 512], BF16, name="masks")
    outT_sb = persist.tile([128, HPC, T], BF16, name="outT")
    if SWAP_MODE == "pe":
        swT = persist.tile([128, 128], BF16, name="swT")

    # ======================= Phase 1: QKV + RoPE ===========================
    with (
        tc.tile_pool(name="p1", bufs=1) as p1,
        tc.tile_pool(name="p1w", bufs=2) as p1w,
        tc.tile_pool(name="p1ps", bufs=3, space="PSUM") as p1ps,
        tc.tile_pool(name="p1sw", bufs=2, space="PSUM") as p1sw,
    ):
        wv = p1.tile([128, KD_N, DL], BF16, name="wv")
        xt = p1.tile([128, KD_N, T], BF16, name="xt")
        cosT = p1.tile([128, T], BF16, name="cosT")
        sinsT = p1.tile([128, T], BF16, name="sinsT")

        def load(eng, dst, off, nelem, pat, **kw):
            eng.dma_start(dst, blob[off:off + nelem].rearrange(pat, **kw))

        wk_src = blob[WK_OFF:WK_OFF + WKQ_N].rearrange(
            "(p h c d) -> p h c d", h=HPC, c=KD_N, d=HD
        )
        wq_src = blob[WQ_OFF:WQ_OFF + WKQ_N].rearrange(
            "(p h c d) -> p h c d", h=HPC, c=KD_N, d=HD
        )
        # both queues split the startup-critical loads: the first unit needs
        # head-0 weights plus x's t range 0:1024 (halves 0 and 1)
        w_first = p1w.tile([128, KD_N, HD], BF16, name="w_sb")
        nc.sync.dma_start(w_first[:], wk_src[:, 0, :, :])
        for half, eng in ((0, nc.sync), (1, nc.scalar), (2, nc.sync),
                          (3, nc.sync)):
            load(eng, xt[:, :, half * 512:(half + 1) * 512],
                 XT_OFF + half * (XT_N // 4), XT_N // 4,
                 "(p c t) -> p c t", c=KD_N, t=512)
        # scalar queue: rope tables now, per-head weights stream in from the
        # loop below, wv/wout/masks are emitted (= enqueued) later
        load(nc.scalar, cosT[:], COST_OFF, COST_N, "(p t) -> p t", t=T)
        load(nc.scalar, sinsT[:], SINS_OFF, SINS_N, "(p t) -> p t", t=T)
        if SWAP_MODE == "pe":
            load(nc.scalar, swT[:], SWT_OFF, SWT_N, "(p d) -> p d", d=128)

        for side, w_src, dst in ((0, wk_src, kT), (1, wq_src, qT)):
            for h in range(HPC):
                if side == 0 and h == 0:
                    w_sb = w_first
                else:
                    w_sb = p1w.tile([128, KD_N, HD], BF16, name="w_sb")
                    nc.scalar.dma_start(w_sb[:], w_src[:, h, :, :])
                for tu in range(4):  # matmul PSUM out must fit one bank
                    ts = slice(tu * 512, (tu + 1) * 512)
                    ps = p1ps.tile([128, 512], F32, name="ps")
                    for c in range(KD_N):
                        nc.tensor.matmul(
                            ps[:], w_sb[:, c, :], xt[:, c, ts],
                            start=(c == 0), stop=(c == KD_N - 1),
                        )
                    # RoPE, transposed layout: out = q*cos + swap(q*sin').
                    # The partition-halves swap is two SBUF-to-SBUF DMAs
                    # (engines can't cross partitions; DMA is address-based)
                    # which keeps every DVE op all-bf16-SBUF (4x mode).
                    q_sb = p1w.tile([128, 512], BF16, name="q_sb")
                    nc.scalar.copy(q_sb[:], ps[:])
                    u = p1w.tile([128, 512], BF16, name="u")
                    nc.vector.tensor_mul(u[:], q_sb[:], sinsT[:, ts])
                    if SWAP_MODE == "dma":
                        qsw = p1w.tile([128, 512], BF16, name="qsw")
                        nc.sync.dma_start(qsw[0:64, :], u[64:128, :])
                        nc.scalar.dma_start(qsw[64:128, :], u[0:64, :])
                    else:
                        qsw = p1sw.tile([128, 512], F32, name="qswp")
                        nc.tensor.matmul(
                            qsw[:], swT[:], u[:], start=True, stop=True
                        )
                    nc.vector.tensor_mul(q_sb[:], q_sb[:], cosT[:, ts])
                    nc.vector.tensor_add(dst[:, h, ts], q_sb[:], qsw[:])
        load(nc.scalar, wv[:], WV_OFF, WV_N, "(p c n) -> p c n", c=KD_N, n=DL)
        load(nc.scalar, wout[:], WOUT_OFF, WOUT_N,
             "(h p n) -> p h n", p=128, n=D)
        load(nc.scalar, masks[:], MASK_OFF, MASK_N, "(p v q) -> p v q", v=4,
             q=512)
        for tt in range(KI_N):
            psv = p1ps.tile([128, 512], F32, name="psv", tag="ps")
            for c in range(KD_N):
                nc.tensor.matmul(
                    psv[:],
                    xt[:, c, tt * 128:(tt + 1) * 128],
                    wv[:, c, :],
                    start=(c == 0), stop=(c == KD_N - 1),
                )
            nc.scalar.copy(v_res[:, tt, :], psv[:])

    if stop_after == "p1":
        return

    # =================== Phase 2+3: attention + out-proj ===================
    with (
        tc.tile_pool(name="p2w", bufs=3) as p2w,
        tc.tile_pool(name="p2s", bufs=2) as p2s,
        tc.tile_pool(name="stps", bufs=2, space="PSUM") as stps,
        tc.tile_pool(name="otps", bufs=2, space="PSUM") as otps,
        tc.tile_pool(name="srps", bufs=1, space="PSUM") as srps,
        tc.tile_pool(name="bcps", bufs=1, space="PSUM") as bcps,
        tc.tile_pool(name="yps", bufs=2, space="PSUM") as yps,
        tc.tile_pool(name="p3w", bufs=2) as p3w,
    ):
        y = io["y"]

        def phase3(qt):
            if stop_after == "p2":
                return
            y_sb = p3w.tile([128, D], BF16, name="y_sb")
            for nch in range(NC_N):
                y_ps = yps.tile([128, 512], F32, name="y_ps")
                for hh in range(HPC):
                    nc.tensor.matmul(
                        y_ps[:],
                        outT_sb[:, hh, qt * 128:(qt + 1) * 128],
                        wout[:, hh, nch * 512:(nch + 1) * 512],
                        start=(hh == 0), stop=(hh == HPC - 1),
                    )
                dst = y_sb[:, nch * 512:(nch + 1) * 512]
                if nch % 2 == 0:
                    nc.vector.tensor_copy(dst, y_ps[:])
                else:
                    nc.scalar.copy(dst, y_ps[:])
            eng = nc.sync if qt % 2 == 0 else nc.scalar
            if stop_after == "yhalf":
                eng.dma_start(
                    y[qt * 128:(qt + 1) * 128, 0:D // 2], y_sb[:, 0:D // 2]
                )
            else:
                eng.dma_start(y[qt * 128:(qt + 1) * 128, :], y_sb[:])

        for idx, qc in enumerate(QC_ORDER):
            n_ki = 4 * qc + 4
            prev = QC_ORDER[idx - 1] if idx >= 1 else None
            for h in range(HPC):
                oT = otps.tile([128, 512], F32, name="oT")
                # fp16 partial-sum accumulators, one per <=8-ki group: all
                # 2-byte SBUF operands keep the adds in the 4x DVE mode, and
                # a group's fp16 rounding stays ~0.1% of the denominator
                n_grp = (n_ki + 7) // 8
                sums = [
                    p2s.tile([128, 512], F16, name=f"sums{g}")
                    for g in range(n_grp)
                ]
                for ki in range(n_ki):
                    st = stps.tile([128, 512], F32, name="st")
                    nc.tensor.matmul(
                        st[:], kT[:, h, ki * 128:(ki + 1) * 128],
                        qT[:, h, qc * 512:(qc + 1) * 512],
                        start=True, stop=True,
                    )
                    pt = p2w.tile([128, 512], BF16, name="pt")
                    nc.scalar.activation(pt[:], st[:], AF.Exp, scale=SCALE)
                    if ki >= 4 * qc:
                        nc.vector.tensor_mul(
                            pt[:], pt[:], masks[:, ki - 4 * qc, :]
                        )
                    sm = sums[ki // 8]
                    if ki % 8 == 0:
                        nc.vector.tensor_copy(sm[:], pt[:])
                    else:
                        nc.vector.tensor_add(sm[:], sm[:], pt[:])
                    nc.tensor.matmul(
                        oT[:], v_res[:, ki, h * 128:(h + 1) * 128], pt[:],
                        start=(ki == 0), stop=(ki == n_ki - 1),
                    )
                sr = srps.tile([1, 512], F32, name="sr")
                for g in range(n_grp):
                    nc.tensor.matmul(
                        sr[:], ones_c, sums[g][:],
                        start=(g == 0), stop=(g == n_grp - 1),
                    )
                recip = p2w.tile([1, 512], F32R, name="recip")
                nc.vector.reciprocal(recip[:], sr[:])
                bc_ps = bcps.tile([128, 512], F32, name="bc")
                nc.tensor.matmul(
                    bc_ps[:], ones_r, recip[:], start=True, stop=True
                )
                bc_sb = p2w.tile([128, 512], F32R, name="bc_sb")
                nc.scalar.copy(bc_sb[:], bc_ps[:])
                nc.vector.tensor_mul(
                    outT_sb[:, h, qc * 512:(qc + 1) * 512], oT[:], bc_sb[:]
                )
                # out-proj of the previous q-chunk, interleaved for PE density
                if prev is not None and h < 2:
                    phase3(4 * prev + 2 * h)
                    phase3(4 * prev + 2 * h + 1)
            if idx == QC_N - 1:
                for qt in range(4 * qc, 4 * qc + 4):
                    phase3(qt)


def build_program(reps=None, tiny_out=False, stop_after=None):
    nc = bass.Bass(enable_partition_id=False)
    io = {}
    io["blob"] = nc.dram_tensor("blob", [BLOB_N], BF16, kind="ExternalInput")
    if tiny_out:
        io["y"] = nc.dram_tensor("y", [T, D], BF16)
        io["probe"] = nc.dram_tensor(
            "probe", [128, 512], BF16, kind="ExternalOutput"
        )
    else:
        io["y"] = nc.dram_tensor("y", [T, D], BF16, kind="ExternalOutput")

    from contextlib import ExitStack

    with tile.TileContext(nc) as tc:
        with nc.allow_low_precision(reason="bf16/f32r matmul pipeline"):
            with ExitStack() as stk:
                if reps is not None:
                    stk.enter_context(tc.For_i(0, reps, 1))
                _emit_body(nc, tc, io, stk, stop_after=stop_after)
                if tiny_out:
                    po = stk.enter_context(tc.tile_pool(name="po", bufs=1))
                    ot = po.tile([128, 512], BF16)
                    nc.any.memset(ot[:], 2.0)
                    nc.sync.dma_start(io["probe"][:], ot[:])

    _split_multi_waits(nc)
    return nc


def host_inputs(x, w_qkv, w_out):
    """Build the 8 per-core input maps from the full problem inputs."""
    import ml_dtypes

    bf = ml_dtypes.bfloat16
    x = np.asarray(x, dtype=np.float32)
    w_qkv = np.asarray(w_qkv, dtype=np.float32)
    w_out = np.asarray(w_out, dtype=np.float32)

    # RoPE caches (match reference._rope_cache), transposed layout
    inv_freq = 1.0 / (
        ROPE_THETA ** (np.arange(0, HD, 2, dtype=np.float32) / HD)
    )
    tpos = np.arange(T, dtype=np.float32)
    freqs = np.outer(tpos, inv_freq)
    emb = np.concatenate([freqs, freqs], axis=1)        # [T, 128]
    cosT = np.ascontiguousarray(np.cos(emb).T).astype(np.float32)
    sinT = np.ascontiguousarray(np.sin(emb).T).astype(np.float32)
    # sinm has rows 0:64 negated (they multiply the swapped-in upper half);
    # pre-swap rows by 64 so the multiply can happen before the swap
    sinmT = sinT.copy()
    sinmT[: HD // 2, :] *= -1.0
    sinsT = np.roll(sinmT, -64, axis=0)

    # multiplicative causal masks, ST layout [k-partition, q-free]:
    # variant v: zero iff qf < kp + 128*v
    kp = np.arange(128)[:, None]
    qf = np.arange(512)[None, :]
    masks = np.stack(
        [(qf >= kp + 128 * v).astype(np.float32) for v in range(4)]
    ).transpose(1, 0, 2)                                # [128, 4, 512]

    swT = np.roll(np.eye(128, dtype=np.float32), 64, axis=0)

    def wkq_layout(w):      # [2048, 512] -> [128, 4, 16, 128] flat
        return np.ascontiguousarray(
            w.reshape(KD_N, 128, HPC, HD).transpose(1, 2, 0, 3)
        ).astype(bf).reshape(-1)

    xT_b = []
    for b in range(B):
        xb = np.ascontiguousarray(x[b].T)               # [D, T]
        chunks = [
            np.ascontiguousarray(
                xb[:, tq * 512:(tq + 1) * 512]
                .reshape(KD_N, 128, 512).transpose(1, 0, 2)
            ).astype(bf).reshape(-1)
            for tq in range(4)
        ]
        xT_b.append(np.concatenate(chunks))

    cos_v = cosT.astype(bf).reshape(-1)
    sins_v = sinsT.astype(bf).reshape(-1)
    masks_v = np.ascontiguousarray(masks).astype(bf).reshape(-1)
    swT_v = swT.astype(bf).reshape(-1)

    in_maps = []
    for c in range(N_CORES):
        b = c // CPG
        g = c % CPG
        hs = slice(g * DL, (g + 1) * DL)
        wq_s = wkq_layout(w_qkv[:, :D][:, hs])
        wk_s = wkq_layout(w_qkv[:, D:2 * D][:, hs])
        wv_s = np.ascontiguousarray(
            w_qkv[:, 2 * D:][:, hs].reshape(KD_N, 128, DL).transpose(1, 0, 2)
        ).astype(bf).reshape(-1)
        w_out_s = np.ascontiguousarray(w_out[hs, :]).astype(bf).reshape(-1)
        blob = np.concatenate(
            [wk_s, xT_b[b], wq_s, cos_v, sins_v, wv_s, w_out_s, masks_v,
             swT_v]
        )
        assert blob.shape[0] == BLOB_N, (blob.shape[0], BLOB_N)
        in_maps.append({"blob": blob})
    return in_maps


_NC_CACHE = {}


def kernel(x, w_qkv, w_out):
    if "nc" not in _NC_CACHE:
        _NC_CACHE["nc"] = build_program()
    nc = _NC_CACHE["nc"]
    in_maps = host_inputs(x, w_qkv, w_out)
    res = run_bass_kernel_spmd(nc, in_maps, list(range(N_CORES)))
    y = np.zeros((B, T, D), dtype=np.float64)
    for c in range(N_CORES):
        y[c // CPG] += res.results[c]["y"].astype(np.float64)
    return y.astype(np.float32)


# revision 3
# speedup vs baseline: 1.0043x; 1.0043x over previous
"""Causal self-attention (RoPE) Trainium2 kernel, v2.

Model: B=2, T=2048, D=2048, 16 heads x 128 head-dim, RoPE theta=1e4.

Sharding (8 cores): cores 0-3 own batch 0, cores 4-7 own batch 1; within a
batch group each core owns 4 heads. Each core computes QKV for its heads,
runs causal attention, and produces a partial output projection (its head
rows of w_out); the host sums 4 partials per batch.

v2 strategy (vs the transpose-heavy v1):
- Q^T/K^T are computed DIRECTLY in [head-dim, T] orientation as W^T @ x^T
  (bf16 weight chunks stationary, x^T streamed 1024 wide) - no PE
  transposes, and bf16 stationary operands load in 1 pass (f32r takes 4).
- RoPE in transposed layout. rotate_half is a partition-halves swap, which
  DVE cannot do (no cross-lane path; the device compiler rejects it), so
  the swap is one PE matmul with a constant permutation. The sin table is
  pre-swapped on the host so the elementwise sin-mul happens BEFORE the
  swap: u = q*sin_preswap (all-bf16 SBUF, 4x DVE mode), swap(u) via PE,
  then one 1x add against the swap's PSUM output.
- Causal mask is multiplicative {0,1}-bf16 applied to exp() output (4x DVE)
  instead of additive -1e6 on f32 PSUM logits (1x DVE).
- Softmax denominators accumulate on DVE (f32); one ones-matmul per
  (head, q-chunk) reduces partitions, reciprocal is broadcast back through
  a rank-1 PE matmul.
- Phase-3 (out-proj) t-tiles are emitted interleaved into the NEXT
  q-chunk's attention loop so PE has dense matmul work while ACT chews
  through exp()s.
- All attention internals bf16: vs a float64 oracle this adds nothing over
  the bf16 input quantization (rel err stays ~3.5e-3).
"""

import sys

sys.path.insert(0, "/opt/trn_rl_repo")

import numpy as np

import concourse.bass as bass
import concourse.mybir as mybir
from concourse import tile
from concourse.bass_utils import run_bass_kernel_spmd

F32 = mybir.dt.float32
F32R = mybir.dt.float32r
BF16 = mybir.dt.bfloat16
F16 = mybir.dt.float16
AF = mybir.ActivationFunctionType

B, T, D = 2, 2048, 2048
H, HD = 16, 128
N_CORES = 8
GROUPS = 2                   # batch groups
CPG = N_CORES // GROUPS      # cores per group (4)
HPC = H // CPG               # heads per core (4)
DL = HPC * HD                # local head dims (512)
ROPE_THETA = 10000.0
SCALE = float(HD) ** -0.5

KD_N = D // 128              # 16 contraction chunks over d_model
TC_N = 2                     # t-chunks of 1024 in phase 1
QC_N = T // 512              # 4 q-chunks of 512
KI_N = T // 128              # 16 k-tiles
NC_N = D // 512              # 4 n-chunks for the output projection

SWAP_MODE = "dma"            # "dma": SBUF->SBUF partition swap; "pe": matmul
QC_ORDER = [0, 1, 2, 3]      # reversed order measured the same on HW

# blob layout (bf16 elements); regions ordered by first use so compute can
# start while later regions are still in flight
WK_OFF = 0
WKQ_N = D * DL                        # [128, 4, 16, 128] per side (h-major)
XT_OFF = WK_OFF + WKQ_N
XT_N = D * T                          # 2 chunks of [128, 16, 1024]
WQ_OFF = XT_OFF + XT_N
COST_OFF = WQ_OFF + WKQ_N
COST_N = HD * T                       # [128, 2048] bf16
SINS_OFF = COST_OFF + COST_N
SINS_N = HD * T                       # pre-swapped sin, bf16
WV_OFF = SINS_OFF + SINS_N
WV_N = D * DL                         # [128, 16, 512]
WOUT_OFF = WV_OFF + WV_N
WOUT_N = DL * D                       # [128, 4, 2048]
MASK_OFF = WOUT_OFF + WOUT_N
MASK_N = 4 * 128 * 512                # [128, 4, 512] {0,1} bf16
SWT_OFF = MASK_OFF + MASK_N
SWT_N = 128 * 128                     # [128, 128] swap permutation
BLOB_N = SWT_OFF + SWT_N


def _split_multi_waits(nc):
    """This container's walrus accepts at most ONE semaphore wait per
    instruction; hoist extra waits onto single-wait NoOps inserted right
    before the instruction on the same engine (sequencers run in order, so
    semantics are unchanged)."""
    n = 0
    for f in nc.m.functions:
        for b in f.blocks:
            il = b.instructions
            if not any(
                i.sync_info is not None and len(i.sync_info.on_wait) > 1
                for i in il
            ):
                continue
            out = []
            for inst in il:
                si = inst.sync_info
                if si is not None and len(si.on_wait) > 1:
                    waits = list(si.on_wait)
                    for w in waits[:-1]:
                        nop = mybir.InstNoOp(
                            name=nc.get_next_instruction_name(), ins=[], outs=[]
                        )
                        nop.engine = inst.engine
                        nop.sync_info = mybir.SyncInfo(on_wait=[w], on_update=[])
                        nc.register_instruction(nop)
                        out.append(nop)
                        n += 1
                    inst.sync_info = mybir.SyncInfo(
                        on_wait=[waits[-1]], on_update=list(si.on_update)
                    )
                out.append(inst)
            il[:] = out
    return n


def _emit_body(nc, tc, io, stk, stop_after=None):
    blob = io["blob"]
    persist = stk.enter_context(tc.tile_pool(name="persist", bufs=1))

    # qT/kT: [head-dim part, head, t] bf16; v_res: [t part, ktile, head*d]
    qT = persist.tile([128, HPC, T], BF16, name="qT")
    kT = persist.tile([128, HPC, T], BF16, name="kT")
    v_res = persist.tile([128, KI_N, DL], BF16, name="v_res")
    ones = persist.tile([128, 128], F32, name="ones")  # f32r memset: no ISA
    nc.vector.memset(ones[:], 1.0)
    ones_r = ones[0:1, 0:128].bitcast(F32R)
    # fp16 ones column: walrus requires matmul operand transfer types to
    # match when either is f32/f32r, so the fp16-sums reduce needs its own
    ones16 = persist.tile([128, 1], F16, name="ones16")
    nc.vector.tensor_copy(ones16[:], ones[0:128, 0:1])  # fp16 memset: no ISA
    ones_c = ones16[0:128, 0:1]
    wout = persist.tile([128, HPC, D], BF16, name="wout")
    masks = persist.tile([128, 4, 512], BF16, name="masks")
    outT_sb = persist.tile([128, HPC, T], BF16, name="outT")
    if SWAP_MODE == "pe":
        swT = persist.tile([128, 128], BF16, name="swT")

    # ======================= Phase 1: QKV + RoPE ===========================
    with (
        tc.tile_pool(name="p1", bufs=1) as p1,
        tc.tile_pool(name="p1w", bufs=2) as p1w,
        tc.tile_pool(name="p1ps", bufs=3, space="PSUM") as p1ps,
        tc.tile_pool(name="p1sw", bufs=2, space="PSUM") as p1sw,
    ):
        wv = p1.tile([128, KD_N, DL], BF16, name="wv")
        xt = p1.tile([128, KD_N, T], BF16, name="xt")
        cosT = p1.tile([128, T], BF16, name="cosT")
        sinsT = p1.tile([128, T], BF16, name="sinsT")

        def load(eng, dst, off, nelem, pat, **kw):
            eng.dma_start(dst, blob[off:off + nelem].rearrange(pat, **kw))

        wk_src = blob[WK_OFF:WK_OFF + WKQ_N].rearrange(
            "(p h c d) -> p h c d", h=HPC, c=KD_N, d=HD
        )
        wq_src = blob[WQ_OFF:WQ_OFF + WKQ_N].rearrange(
            "(p h c d) -> p h c d", h=HPC, c=KD_N, d=HD
        )
        # both queues split the startup-critical loads: the first unit needs
        # head-0 weights plus x's t range 0:1024 (halves 0 and 1)
        w_first = p1w.tile([128, KD_N, HD], BF16, name="w_sb")
        nc.sync.dma_start(w_first[:], wk_src[:, 0, :, :])
        for half, eng in ((0, nc.sync), (1, nc.scalar), (2, nc.sync),
                          (3, nc.sync)):
            load(eng, xt[:, :, half * 512:(half + 1) * 512],
                 XT_OFF + half * (XT_N // 4), XT_N // 4,
                 "(p c t) -> p c t", c=KD_N, t=512)
        # scalar queue: rope tables now, per-head weights stream in from the
        # loop below, wv/wout/masks are emitted (= enqueued) later
        load(nc.scalar, cosT[:], COST_OFF, COST_N, "(p t) -> p t", t=T)
        load(nc.scalar, sinsT[:], SINS_OFF, SINS_N, "(p t) -> p t", t=T)
        if SWAP_MODE == "pe":
            load(nc.scalar, swT[:], SWT_OFF, SWT_N, "(p d) -> p d", d=128)

        for side, w_src, dst in ((0, wk_src, kT), (1, wq_src, qT)):
            for h in range(HPC):
                if side == 0 and h == 0:
                    w_sb = w_first
                else:
                    w_sb = p1w.tile([128, KD_N, HD], BF16, name="w_sb")
                    nc.scalar.dma_start(w_sb[:], w_src[:, h, :, :])
                for tu in range(4):  # matmul PSUM out must fit one bank
                    ts = slice(tu * 512, (tu + 1) * 512)
                    ps = p1ps.tile([128, 512], F32, name="ps")
                    for c in range(KD_N):
                        nc.tensor.matmul(
                            ps[:], w_sb[:, c, :], xt[:, c, ts],
                            start=(c == 0), stop=(c == KD_N - 1),
                        )
                    # RoPE, transposed layout: out = q*cos + swap(q*sin').
                    # The partition-halves swap is two SBUF-to-SBUF DMAs
                    # (engines can't cross partitions; DMA is address-based)
                    # which keeps every DVE op all-bf16-SBUF (4x mode).
                    q_sb = p1w.tile([128, 512], BF16, name="q_sb")
                    nc.scalar.copy(q_sb[:], ps[:])
                    u = p1w.tile([128, 512], BF16, name="u")
                    nc.vector.tensor_mul(u[:], q_sb[:], sinsT[:, ts])
                    if SWAP_MODE == "dma":
                        qsw = p1w.tile([128, 512], BF16, name="qsw")
                        nc.sync.dma_start(qsw[0:64, :], u[64:128, :])
                        nc.scalar.dma_start(qsw[64:128, :], u[0:64, :])
                    else:
                        qsw = p1sw.tile([128, 512], F32, name="qswp")
                        nc.tensor.matmul(
                            qsw[:], swT[:], u[:], start=True, stop=True
                        )
                    nc.vector.tensor_mul(q_sb[:], q_sb[:], cosT[:, ts])
                    nc.vector.tensor_add(dst[:, h, ts], q_sb[:], qsw[:])
        load(nc.scalar, wv[:], WV_OFF, WV_N, "(p c n) -> p c n", c=KD_N, n=DL)
        load(nc.scalar, wout[:], WOUT_OFF, WOUT_N,
             "(h p n) -> p h n", p=128, n=D)
        load(nc.scalar, masks[:], MASK_OFF, MASK_N, "(p v q) -> p v q", v=4,
             q=512)
        for tt in range(KI_N):
            psv = p1ps.tile([128, 512], F32, name="psv", tag="ps")
            for c in range(KD_N):
                nc.tensor.matmul(
                    psv[:],
                    xt[:, c, tt * 128:(tt + 1) * 128],
                    wv[:, c, :],
                    start=(c == 0), stop=(c == KD_N - 1),
                )
            nc.scalar.copy(v_res[:, tt, :], psv[:])

    if stop_after == "p1":
        return

    # =================== Phase 2+3: attention + out-proj ===================
    with (
        tc.tile_pool(name="p2w", bufs=3) as p2w,
        tc.tile_pool(name="p2s", bufs=2) as p2s,
        tc.tile_pool(name="stps", bufs=2, space="PSUM") as stps,
        tc.tile_pool(name="otps", bufs=2, space="PSUM") as otps,
        tc.tile_pool(name="srps", bufs=1, space="PSUM") as srps,
        tc.tile_pool(name="bcps", bufs=1, space="PSUM") as bcps,
        tc.tile_pool(name="yps", bufs=2, space="PSUM") as yps,
        tc.tile_pool(name="p3w", bufs=2) as p3w,
    ):
        y = io["y"]

        # phase-3 out-proj, decomposed into (qt, nch) units so they can be
        # interleaved into the attention ki-loops at fine grain: ACT paces
        # each ki (exp 0.61us vs 0.43us of PE work) and the PE queue is
        # in-order, so only work emitted BETWEEN STs can fill those bubbles
        p3_queue = []
        p3_open = {}

        def p3_unit():
            if not p3_queue:
                return
            qt, nch = p3_queue.pop(0)
            if stop_after == "p2":
                return
            if qt not in p3_open:
                p3_open[qt] = p3w.tile([128, D], BF16, name="y_sb")
            y_sb = p3_open[qt]
            y_ps = yps.tile([128, 512], F32, name="y_ps")
            for hh in range(HPC):
                nc.tensor.matmul(
                    y_ps[:],
                    outT_sb[:, hh, qt * 128:(qt + 1) * 128],
                    wout[:, hh, nch * 512:(nch + 1) * 512],
                    start=(hh == 0), stop=(hh == HPC - 1),
                )
            dst = y_sb[:, nch * 512:(nch + 1) * 512]
            if nch % 2 == 0:
                nc.vector.tensor_copy(dst, y_ps[:])
            else:
                nc.scalar.copy(dst, y_ps[:])
            if nch == NC_N - 1:
                del p3_open[qt]
                eng = nc.sync if qt % 2 == 0 else nc.scalar
                eng.dma_start(y[qt * 128:(qt + 1) * 128, :], y_sb[:])

        for idx, qc in enumerate(QC_ORDER):
            n_ki = 4 * qc + 4
            prev = QC_ORDER[idx - 1] if idx >= 1 else None
            if prev is not None:
                p3_queue.extend(
                    (4 * prev + i, nch) for i in range(4) for nch in range(NC_N)
                )
            for h in range(HPC):
                oT = otps.tile([128, 512], F32, name="oT")
                # fp16 partial-sum accumulators, one per <=8-ki group: all
                # 2-byte SBUF operands keep the adds in the 4x DVE mode, and
                # a group's fp16 rounding stays ~0.1% of the denominator
                n_grp = (n_ki + 7) // 8
                sums = [
                    p2s.tile([128, 512], F16, name=f"sums{g}")
                    for g in range(n_grp)
                ]
                for ki in range(n_ki):
                    st = stps.tile([128, 512], F32, name="st")
                    nc.tensor.matmul(
                        st[:], kT[:, h, ki * 128:(ki + 1) * 128],
                        qT[:, h, qc * 512:(qc + 1) * 512],
                        start=True, stop=True,
                    )
                    pt = p2w.tile([128, 512], BF16, name="pt")
                    nc.scalar.activation(pt[:], st[:], AF.Exp, scale=SCALE)
                    if ki >= 4 * qc:
                        nc.vector.tensor_mul(
                            pt[:], pt[:], masks[:, ki - 4 * qc, :]
                        )
                    sm = sums[ki // 8]
                    if ki % 8 == 0:
                        nc.vector.tensor_copy(sm[:], pt[:])
                    else:
                        nc.vector.tensor_add(sm[:], sm[:], pt[:])
                    nc.tensor.matmul(
                        oT[:], v_res[:, ki, h * 128:(h + 1) * 128], pt[:],
                        start=(ki == 0), stop=(ki == n_ki - 1),
                    )
                    if ki % 2 == 1:
                        p3_unit()
                sr = srps.tile([1, 512], F32, name="sr")
                for g in range(n_grp):
                    nc.tensor.matmul(
                        sr[:], ones_c, sums[g][:],
                        start=(g == 0), stop=(g == n_grp - 1),
                    )
                recip = p2w.tile([1, 512], F32R, name="recip")
                nc.vector.reciprocal(recip[:], sr[:])
                bc_ps = bcps.tile([128, 512], F32, name="bc")
                nc.tensor.matmul(
                    bc_ps[:], ones_r, recip[:], start=True, stop=True
                )
                bc_sb = p2w.tile([128, 512], F32R, name="bc_sb")
                nc.scalar.copy(bc_sb[:], bc_ps[:])
                nc.vector.tensor_mul(
                    outT_sb[:, h, qc * 512:(qc + 1) * 512], oT[:], bc_sb[:]
                )
                p3_unit()
            while p3_queue:
                p3_unit()
            if idx == QC_N - 1:
                p3_queue.extend(
                    (4 * qc + i, nch) for i in range(4) for nch in range(NC_N)
                )
                while p3_queue:
                    p3_unit()


def build_program(reps=None, tiny_out=False, stop_after=None):
    nc = bass.Bass(enable_partition_id=False)
    io = {}
    io["blob"] = nc.dram_tensor("blob", [BLOB_N], BF16, kind="ExternalInput")
    if tiny_out:
        io["y"] = nc.dram_tensor("y", [T, D], BF16)
        io["probe"] = nc.dram_tensor(
            "probe", [128, 512], BF16, kind="ExternalOutput"
        )
    else:
        io["y"] = nc.dram_tensor("y", [T, D], BF16, kind="ExternalOutput")

    from contextlib import ExitStack

    with tile.TileContext(nc) as tc:
        with nc.allow_low_precision(reason="bf16/f32r matmul pipeline"):
            with ExitStack() as stk:
                if reps is not None:
                    stk.enter_context(tc.For_i(0, reps, 1))
                _emit_body(nc, tc, io, stk, stop_after=stop_after)
                if tiny_out:
                    po = stk.enter_context(tc.tile_pool(name="po", bufs=1))
                    ot = po.tile([128, 512], BF16)
                    nc.any.memset(ot[:], 2.0)
                    nc.sync.dma_start(io["probe"][:], ot[:])

    _split_multi_waits(nc)
    return nc


def host_inputs(x, w_qkv, w_out):
    """Build the 8 per-core input maps from the full problem inputs."""
    import ml_dtypes

    bf = ml_dtypes.bfloat16
    x = np.asarray(x, dtype=np.float32)
    w_qkv = np.asarray(w_qkv, dtype=np.float32)
    w_out = np.asarray(w_out, dtype=np.float32)

    # RoPE caches (match reference._rope_cache), transposed layout
    inv_freq = 1.0 / (
        ROPE_THETA ** (np.arange(0, HD, 2, dtype=np.float32) / HD)
    )
    tpos = np.arange(T, dtype=np.float32)
    freqs = np.outer(tpos, inv_freq)
    emb = np.concatenate([freqs, freqs], axis=1)        # [T, 128]
    cosT = np.ascontiguousarray(np.cos(emb).T).astype(np.float32)
    sinT = np.ascontiguousarray(np.sin(emb).T).astype(np.float32)
    # sinm has rows 0:64 negated (they multiply the swapped-in upper half);
    # pre-swap rows by 64 so the multiply can happen before the swap
    sinmT = sinT.copy()
    sinmT[: HD // 2, :] *= -1.0
    sinsT = np.roll(sinmT, -64, axis=0)

    # multiplicative causal masks, ST layout [k-partition, q-free]:
    # variant v: zero iff qf < kp + 128*v
    kp = np.arange(128)[:, None]
    qf = np.arange(512)[None, :]
    masks = np.stack(
        [(qf >= kp + 128 * v).astype(np.float32) for v in range(4)]
    ).transpose(1, 0, 2)                                # [128, 4, 512]

    swT = np.roll(np.eye(128, dtype=np.float32), 64, axis=0)

    def wkq_layout(w):      # [2048, 512] -> [128, 4, 16, 128] flat
        return np.ascontiguousarray(
            w.reshape(KD_N, 128, HPC, HD).transpose(1, 2, 0, 3)
        ).astype(bf).reshape(-1)

    xT_b = []
    for b in range(B):
        xb = np.ascontiguousarray(x[b].T)               # [D, T]
        chunks = [
            np.ascontiguousarray(
                xb[:, tq * 512:(tq + 1) * 512]
                .reshape(KD_N, 128, 512).transpose(1, 0, 2)
            ).astype(bf).reshape(-1)
            for tq in range(4)
        ]
        xT_b.append(np.concatenate(chunks))

    cos_v = cosT.astype(bf).reshape(-1)
    sins_v = sinsT.astype(bf).reshape(-1)
    masks_v = np.ascontiguousarray(masks).astype(bf).reshape(-1)
    swT_v = swT.astype(bf).reshape(-1)

    in_maps = []
    for c in range(N_CORES):
        b = c // CPG
        g = c % CPG
        hs = slice(g * DL, (g + 1) * DL)
        wq_s = wkq_layout(w_qkv[:, :D][:, hs])
        wk_s = wkq_layout(w_qkv[:, D:2 * D][:, hs])
        wv_s = np.ascontiguousarray(
            w_qkv[:, 2 * D:][:, hs].reshape(KD_N, 128, DL).transpose(1, 0, 2)
        ).astype(bf).reshape(-1)
        w_out_s = np.ascontiguousarray(w_out[hs, :]).astype(bf).reshape(-1)
        blob = np.concatenate(
            [wk_s, xT_b[b], wq_s, cos_v, sins_v, wv_s, w_out_s, masks_v,
             swT_v]
        )
        assert blob.shape[0] == BLOB_N, (blob.shape[0], BLOB_N)
        in_maps.append({"blob": blob})
    return in_maps


_NC_CACHE = {}


def kernel(x, w_qkv, w_out):
    if "nc" not in _NC_CACHE:
        _NC_CACHE["nc"] = build_program()
    nc = _NC_CACHE["nc"]
    in_maps = host_inputs(x, w_qkv, w_out)
    res = run_bass_kernel_spmd(nc, in_maps, list(range(N_CORES)))
    y = np.zeros((B, T, D), dtype=np.float64)
    for c in range(N_CORES):
        y[c // CPG] += res.results[c]["y"].astype(np.float64)
    return y.astype(np.float32)
